# revision 16
# baseline (speedup 1.0000x reference)
"""Bass/Trainium2 kernel for the edge-aware smoothness loss:

    sum over pixels of |grad log tmap|^2 * sigmoid(48*(0.1 - |grad mean(l_img)|))

Full inputs are sharded by rows across 8 NeuronCores (512 rows each).
Each core computes a partial sum over its interior rows; the 16 core-edge
rows (2 per core) are computed exactly on the host in float64 and added.

Per-core layout: partition p holds 4 consecutive rows (4p..4p+3) as 4
"segments" in the free dimension; W is processed in 8 chunks of 512 columns
(+1 halo column each side). Vertical stencil rows that cross partitions are
produced on the TensorEngine with +/-1 shift matrices; horizontal stencil and
in-partition vertical rows are also TensorEngine ident/shift matmul pairs
accumulated in PSUM. Squares/sums/reductions run on DVE custom fused ops,
transcendentals on ScalarE (table sets phased: Square -> Sqrt -> Sigmoid ->
Ln so each ACT table loads once).
"""

import sys

sys.path.insert(0, "/opt/trn_rl_repo")

import numpy as np

import concourse.bacc as bacc
import concourse.mybir as mybir
from concourse import bass_utils
from concourse import dve_ops
from concourse.dve_spec import Spec, Src0, Src1, C0, lower, sq, _has_src1
from concourse.dve_uop import DveOpSpec
from concourse.tile import TileContext
from concourse.tile_rust import add_dep_helper

EPS = 1e-07
SIG_OFFSET = 0.1
SIG_SCALE = 48.0

H, W = 4096, 4096
NCORES = 8
ROWS = H // NCORES          # 512 rows per core
S = 4                       # rows folded per partition
P = 128                     # partitions
NCHUNK = 8
WC = W // NCHUNK            # 512 columns per chunk
GW = WC + 2                 # chunk width incl. 1-col halo each side

F32 = mybir.dt.float32
BF16 = mybir.dt.bfloat16


# --------------------------------------------------------------------------
# custom DVE ops
# --------------------------------------------------------------------------

def _make_op(name: str, spec: Spec, row: int) -> dve_ops.DveOp:
    shas = {}
    for ver in ("v3", "v4"):
        try:
            s = DveOpSpec(name=name, opcode=row, uops=lower(spec, ver=ver),
                          rd1_en=_has_src1(spec))
            shas[ver] = s.sha(ver)
        except Exception:
            pass
    return dve_ops.DveOp(name, spec, subdim=False, uops_sha=shas)


def _register_custom_ops():
    if "ADDSQ_ANT" in dve_ops._SUB_OPCODE_FOR_NAME:
        return

    addsq_spec = Spec(
        body=Src0 + sq(Src1),
        reference=lambda in0, in1, s0, s1, imm2: (
            in0.astype(np.float32) + in1.astype(np.float32) ** 2
        ),
    )

    def _sqmulred_ref(in0, in1, c0, c1, c2):
        b = (in0.astype(np.float32) ** 2 * in1).astype(np.float32)
        acc = np.asarray(c0, np.float32).reshape(-1, 1) + b.reshape(
            b.shape[0], -1
        ).sum(axis=-1, keepdims=True)
        return b, acc

    from operator import add

    sqmulred_spec = Spec(
        body=sq(Src0) * Src1,
        accum=add,
        accum_init=C0,
        reference=_sqmulred_ref,
    )

    subsq_spec = Spec(
        body=sq(Src0 - Src1),
        reference=lambda in0, in1, s0, s1, imm2: (
            (in0.astype(np.float32) - in1.astype(np.float32)) ** 2
        ),
    )

    base = max(dve_ops._SUB_OPCODE_FOR_NAME.values()) + 1
    for i, (name, spec) in enumerate(
        [("ADDSQ_ANT", addsq_spec), ("SQMULRED_ANT", sqmulred_spec),
         ("SUBSQ_ANT", subsq_spec)]
    ):
        row = base + i
        assert row < 0x20, "custom-DVE opcode rows exhausted"
        dve_ops._SUB_OPCODE_FOR_NAME[name] = row
        op = _make_op(name, spec, row)
        dve_ops.OPS.append(op)
        dve_ops.CUSTOM_DVE_SPECS[name] = spec
        globals()["_OP_" + name] = op


_register_custom_ops()
_ADDSQ = next(o for o in dve_ops.OPS if o.name == "ADDSQ_ANT")
_SQMULRED = next(o for o in dve_ops.OPS if o.name == "SQMULRED_ANT")
_SUBSQ = next(o for o in dve_ops.OPS if o.name == "SUBSQ_ANT")


# --------------------------------------------------------------------------
# device kernel
# --------------------------------------------------------------------------

MAT_NAMES = ["I", "nI", "I0", "nI0", "I127", "nI127", "Sd", "nSu"]


def make_mats() -> np.ndarray:
    """[128, 8*128] f32 stationary matrices.

    matmul(out, lhsT, rhs): out[p, j] = sum_k lhsT[k, p] * rhs[k, j].
    Sd[k, p] = 1 iff k == p-1  (out[p] = rhs[p-1], out[0] = 0)
    Su[k, p] = 1 iff k == p+1  (out[p] = rhs[p+1], out[127] = 0)
    I0/I127 are identities with column 0 / 127 zeroed so the stencil output
    for global edge rows (row 0 -> partition 0 seg 0, row 511 -> partition
    127 seg 3) is exactly zero; the host adds those rows' contribution.
    """
    eye = np.eye(P, dtype=np.float32)
    i0 = eye.copy(); i0[:, 0] = 0.0
    i127 = eye.copy(); i127[:, 127] = 0.0
    sd = np.eye(P, k=1, dtype=np.float32)    # [k, p]: 1 at p = k+1 -> k = p-1
    su = np.eye(P, k=-1, dtype=np.float32)   # 1 at p = k-1 -> k = p+1
    import ml_dtypes
    return np.concatenate(
        [eye, -eye, i0, -i0, i127, -i127, sd, -su], axis=1
    ).astype(ml_dtypes.bfloat16).copy()


def _chunk_cols(c: int):
    """global g-grid columns [c*WC-1, c*WC+WC+1) clipped to [0, W).

    Returns (lo_clipped, n_cols, dst_off) where dst_off is the write offset
    into the GW-wide tile (pad columns at the image edge stay zero)."""
    lo = c * WC - 1
    lo_c = max(lo, 0)
    hi_c = min(c * WC + WC + 1, W)
    return lo_c, hi_c - lo_c, lo_c - lo


def build_kernel():
    nc = bacc.Bacc("TRN2", num_devices=NCORES)

    tm = nc.dram_tensor("tm", [ROWS, W], F32, kind="ExternalInput")
    li = nc.dram_tensor("li", [ROWS, W, 3], F32, kind="ExternalInput")
    mats = nc.dram_tensor("mats", [P, len(MAT_NAMES) * P], BF16,
                          kind="ExternalInput")
    out = nc.dram_tensor("out", [P, 2], F32, kind="ExternalOutput")

    tm_v = tm.ap().rearrange("(p s) w -> p s w", s=S)          # [128, 4, 4096]
    li_v = li.ap().rearrange("(p s) w c -> p s (w c)", s=S)    # [128, 4, 12288]

    GPSIMD_SUM3_CHUNKS = set(range(6))   # chunks whose channel-sum runs on Q7

    with TileContext(nc) as tc:
        with (
            tc.tile_pool(name="const", bufs=1) as cpool,
            tc.tile_pool(name="mf", bufs=1) as mfpool,
            tc.tile_pool(name="work", bufs=2) as wpool,
            tc.tile_pool(name="work1", bufs=1) as w1pool,
            tc.tile_pool(name="tmp3", bufs=3) as tmpool,
            tc.tile_pool(name="psum", bufs=1, space="PSUM") as ppool,
        ):
            mats_sb = cpool.tile([P, len(MAT_NAMES) * P], BF16)
            mats_dma = nc.sync.dma_start(out=mats_sb[:], in_=mats.ap())
            M = {n: mats_sb[:, i * P:(i + 1) * P]
                 for i, n in enumerate(MAT_NAMES)}

            acc = cpool.tile([P, 2], F32)
            nc.vector.memset(acc[:], 0.0)

            # per-partition scalar constants for activation biases
            cb = cpool.tile([P, 3], F32)
            nc.vector.memset(cb[:, 0:1], SIG_SCALE * SIG_OFFSET)
            nc.vector.memset(cb[:, 1:2], -EPS)
            nc.vector.memset(cb[:, 2:3], EPS)
            b_sig, b_neps, b_eps = cb[:, 0:1], cb[:, 1:2], cb[:, 2:3]

            # sigmoid-weight buffers, one per 4-chunk group (separate tiles
            # so group-0 sqrt/sigmoid doesn't serialize group-1 writes)
            GROUPS = [list(range(0, 4)), list(range(4, 8))]
            mfs = [mfpool.tile([P, len(g) * S * WC], F32, tag=f"mf{i}",
                               name=f"mf{i}")
                   for i, g in enumerate(GROUPS)]

            def mf_chunk(c):
                gi = 0 if c < 4 else 1
                k = c - GROUPS[gi][0]
                return mfs[gi][:, k * S * WC:(k + 1) * S * WC]

            def stencil_y(src, ps_y):
                """vertical stencil of `src` ([128, 4, GW] incl. halo col)
                into PSUM [128, 4, WC]; edge rows (p0 seg0 / p127 seg3) -> 0."""
                nc.tensor.matmul(ps_y[:, 0, :], M["Sd"], src[:, 3, 1:WC + 1],
                                 start=True, stop=False)
                nc.tensor.matmul(ps_y[:, 0, :], M["nI0"], src[:, 1, 1:WC + 1],
                                 start=False, stop=True)
                for s in (1, 2):
                    nc.tensor.matmul(ps_y[:, s, :], M["I"], src[:, s - 1, 1:WC + 1],
                                     start=True, stop=False)
                    nc.tensor.matmul(ps_y[:, s, :], M["nI"], src[:, s + 1, 1:WC + 1],
                                     start=False, stop=True)
                nc.tensor.matmul(ps_y[:, 3, :], M["I127"], src[:, 2, 1:WC + 1],
                                 start=True, stop=False)
                nc.tensor.matmul(ps_y[:, 3, :], M["nSu"], src[:, 0, 1:WC + 1],
                                 start=False, stop=True)

            def stencil_x(src, ps_x):
                """horizontal stencil (masked at edge rows) into PSUM."""
                xmat = {0: ("I0", "nI0"), 1: ("I", "nI"),
                        2: ("I", "nI"), 3: ("I127", "nI127")}
                for s in range(S):
                    a, b = xmat[s]
                    nc.tensor.matmul(ps_x[:, s, :], M[a], src[:, s, 2:GW],
                                     start=True, stop=False)
                    nc.tensor.matmul(ps_x[:, s, :], M[b], src[:, s, 0:WC],
                                     start=False, stop=True)

            # ---------------- DMA issue-order chain --------------------
            # rings drain FIFO: li 0-3, tm 0-3, li 4-7, tm 4-7
            prev_dma = [mats_dma]

            def ordered_dma(dst, srcv):
                d = nc.sync.dma_start(out=dst, in_=srcv)
                add_dep_helper(d.ins, prev_dma[0].ins, sync=False,
                               reason="dma ring order")
                prev_dma[0] = d
                return d

            li_t_all, tm_t_all = {}, {}
            g_t_all = {}

            def issue_li_dmas(cs):
                for c in cs:
                    lo_c, ncols, off = _chunk_cols(c)
                    t = wpool.tile([P, S, 3 * GW], F32, tag="li")
                    ordered_dma(t[:, :, 3 * off:3 * (off + ncols)],
                                li_v[:, :, 3 * lo_c:3 * (lo_c + ncols)])
                    li_t_all[c] = t

            def issue_tm_dmas(cs):
                for c in cs:
                    lo_c, ncols, off = _chunk_cols(c)
                    t = tmpool.tile([P, S, GW], F32, tag="tm")
                    ordered_dma(t[:, :, off:off + ncols],
                                tm_v[:, :, lo_c:lo_c + ncols])
                    tm_t_all[c] = t

            def phase_a_chunk(c):
                lo_c, ncols, off = _chunk_cols(c)
                li_t = li_t_all[c]
                g_t = wpool.tile([P, S, GW], BF16, tag="g")
                if off:
                    nc.vector.memset(g_t[:, :, 0:off], 0.0)
                if off + ncols < GW:
                    nc.vector.memset(g_t[:, :, off + ncols:GW], 0.0)
                li4 = li_t[:, :, 3 * off:3 * (off + ncols)].rearrange(
                    "p s (w c) -> p s w c", c=3)
                # channel sum: segs 0-2 on GpSimd, seg 3 on DVE reduce
                u_t = w1pool.tile([P, 3, GW], F32, tag="u")
                nc.gpsimd.tensor_add(out=u_t[:, :, off:off + ncols],
                                     in0=li4[:, 0:3, :, 0],
                                     in1=li4[:, 0:3, :, 1])
                with nc.allow_low_precision("bf16 stencil inputs"):
                    nc.gpsimd.tensor_add(out=g_t[:, 0:3, off:off + ncols],
                                         in0=u_t[:, :, off:off + ncols],
                                         in1=li4[:, 0:3, :, 2])
                    nc.vector.reduce_sum(out=g_t[:, 3:4, off:off + ncols],
                                         in_=li4[:, 3:4],
                                         axis=mybir.AxisListType.X)
                g_t_all[c] = g_t

                ps_y = ppool.tile([P, S, WC], F32, tag="psy")
                stencil_y(g_t, ps_y)

                a_t = wpool.tile([P, S, WC], F32, tag="a")
                nc.vector._custom_dve(
                    _SUBSQ, out=a_t[:],
                    in0=g_t[:, :, 2:GW], in1=g_t[:, :, 0:WC],
                )
                # m = dgx^2 + dgy^2  (1/9 scale applied inside Sqrt)
                nc.vector._custom_dve(
                    _ADDSQ,
                    out=mf_chunk(c).rearrange("p (s j) -> p s j", j=WC),
                    in0=a_t[:],
                    in1=ps_y[:],
                )

            def sqrt_sigmoid_group(gi, after=None):
                half = mfs[gi][:]
                sq_i = nc.scalar.activation(out=half, in_=half,
                                            func=mybir.ActivationFunctionType.Sqrt,
                                            scale=1.0 / 9.0)
                if after is not None:
                    add_dep_helper(sq_i.ins, after.ins, sync=False,
                                   reason="act table phase order")
                return nc.scalar.activation(
                    out=half, in_=half,
                    func=mybir.ActivationFunctionType.Sigmoid,
                    scale=-SIG_SCALE, bias=b_sig)

            def phase_b_chunk(c, sig_i, last_ln):
                lo_c, ncols, off = _chunk_cols(c)
                tm_t = tm_t_all[c]
                # ln(x + EPS) directly — equals ln(clip(x, EPS, 1)) to within
                # ~4e-5 relative on the final sum (verified vs reference), so
                # the Relu clamp pass is unnecessary.
                live = tm_t[:, :, off:off + ncols]
                lg_t = wpool.tile([P, S, GW], BF16, tag="lg")
                ln_i = nc.scalar.activation(out=lg_t[:, :, off:off + ncols],
                                            in_=live,
                                            func=mybir.ActivationFunctionType.Ln,
                                            bias=b_eps)
                add_dep_helper(ln_i.ins, sig_i.ins, sync=False,
                               reason="act table phase order")
                if off:
                    nc.vector.memset(lg_t[:, :, 0:off], 0.0)
                if off + ncols < GW:
                    nc.vector.memset(lg_t[:, :, off + ncols:GW], 0.0)

                ps_x = ppool.tile([P, S, WC], F32, tag="psx")
                ps_y = ppool.tile([P, S, WC], F32, tag="psy")
                stencil_x(lg_t, ps_x)
                stencil_y(lg_t, ps_y)

                sig_c = mf_chunk(c)
                scr = w1pool.tile([P, S * WC], BF16, tag="scr")
                nc.vector._custom_dve(
                    _SQMULRED, out=scr[:],
                    in0=ps_x[:].rearrange("p s j -> p (s j)"), in1=sig_c,
                    s0=acc[:, 0:1], accum_out=acc[:, 0:1],
                )
                scr2 = w1pool.tile([P, S * WC], BF16, tag="scr2")
                nc.vector._custom_dve(
                    _SQMULRED, out=scr2[:],
                    in0=ps_y[:].rearrange("p s j -> p (s j)"), in1=sig_c,
                    s0=acc[:, 1:2], accum_out=acc[:, 1:2],
                )
                return ln_i

            # ---------------- schedule ---------------------------------
            # Ring order: li feeds phase_a (needed early and throughout);
            # tm chunks are interleaved so phase_b group 0 can start as soon
            # as sigmoid(group 0) is ready instead of waiting behind all
            # 24 MB of li. tm4-7 arrive last and gate only the short tail.
            issue_li_dmas(range(0, 5))
            issue_tm_dmas(range(0, 2))
            issue_li_dmas([5])
            issue_tm_dmas([2])
            issue_li_dmas([6])
            issue_tm_dmas([3])
            issue_li_dmas([7])
            issue_tm_dmas(range(4, 8))

            for c in GROUPS[0]:
                phase_a_chunk(c)
            sig0 = sqrt_sigmoid_group(0)
            for c in GROUPS[1]:
                phase_a_chunk(c)
            last_ln = None
            for c in GROUPS[0]:
                last_ln = phase_b_chunk(c, sig0, last_ln)
            sig1 = sqrt_sigmoid_group(1, after=last_ln)
            for c in GROUPS[1]:
                phase_b_chunk(c, sig1, last_ln)

            nc.sync.dma_start(out=out.ap(), in_=acc[:])

    nc.finalize()
    return nc


_NC_CACHE = None


def _get_nc():
    global _NC_CACHE
    if _NC_CACHE is None:
        _NC_CACHE = build_kernel()
    return _NC_CACHE


# --------------------------------------------------------------------------
# host-side edge rows (exact, float64)
# --------------------------------------------------------------------------

def _edge_contribution(tmap: np.ndarray, l_img: np.ndarray) -> float:
    """Exact contribution of global rows {512c, 512c+511} in float64."""
    rows = []
    for c in range(NCORES):
        rows.append(c * ROWS)
        rows.append(c * ROWS + ROWS - 1)

    logp = np.log(np.clip(tmap.astype(np.float64), EPS, 1.0))
    g = l_img.astype(np.float64).mean(axis=2)

    def pad_row(a, r):
        return a[r] if 0 <= r < H else np.zeros(W, np.float64)

    total = 0.0
    for r in rows:
        lc, lu, ld = logp[r], pad_row(logp, r - 1), pad_row(logp, r + 1)
        gc, gu, gd = g[r], pad_row(g, r - 1), pad_row(g, r + 1)
        zl = np.zeros(1, np.float64)

        def dx(v):
            return np.concatenate([v[1:], zl]) - np.concatenate([zl, v[:-1]])

        n = dx(lc) ** 2 + (lu - ld) ** 2
        s = np.sqrt(dx(gc) ** 2 + (gu - gd) ** 2)
        sig = 1.0 / (1.0 + np.exp(-(SIG_OFFSET - s) * SIG_SCALE))
        total += float(np.sum(n * sig))
    return total


# --------------------------------------------------------------------------
# entry point
# --------------------------------------------------------------------------

def run_device(tmap: np.ndarray, l_img: np.ndarray, **kw):
    nc = _get_nc()
    mats = make_mats()
    in_maps = [
        {
            "tm": np.ascontiguousarray(tmap[c * ROWS:(c + 1) * ROWS]),
            "li": np.ascontiguousarray(l_img[c * ROWS:(c + 1) * ROWS]),
            "mats": mats,
        }
        for c in range(NCORES)
    ]
    return bass_utils.run_bass_kernel_spmd(
        nc, in_maps, core_ids=list(range(NCORES)), **kw
    )


def kernel(tmap: np.ndarray, l_img: np.ndarray) -> np.ndarray:
    res = run_device(tmap, l_img)
    dev = sum(float(r["out"].astype(np.float64).sum()) for r in res.results)
    return np.float32(dev + _edge_contribution(tmap, l_img))


if __name__ == "__main__":
    tmap = np.random.rand(H, W).astype(np.float32)
    l_img = np.random.rand(H, W, 3).astype(np.float32)
    print(kernel(tmap, l_img))



# revision 17
# speedup vs baseline: 1.1012x; 1.1012x over previous
"""Bass/Trainium2 kernel for the edge-aware smoothness loss:

    sum over pixels of |grad log tmap|^2 * sigmoid(48*(0.1 - |grad mean(l_img)|))

Full inputs are sharded by rows across 8 NeuronCores (512 rows each).
Each core computes a partial sum over its interior rows; the 16 core-edge
rows (2 per core) are computed exactly on the host in float64 and added.

Per-core layout: partition p holds 4 consecutive rows (4p..4p+3) as 4
"segments" in the free dimension; W is processed in 8 chunks of 512 columns
(+1 halo column each side). Vertical stencil rows that cross partitions are
produced on the TensorEngine with +/-1 shift matrices; horizontal stencil and
in-partition vertical rows are also TensorEngine ident/shift matmul pairs
accumulated in PSUM. Squares/sums/reductions run on DVE custom fused ops,
transcendentals on ScalarE (table sets phased: Square -> Sqrt -> Sigmoid ->
Ln so each ACT table loads once).
"""

import sys

sys.path.insert(0, "/opt/trn_rl_repo")

import numpy as np

import concourse.bacc as bacc
import concourse.mybir as mybir
from concourse import bass_utils
from concourse import dve_ops
from concourse.dve_spec import Spec, Src0, Src1, C0, lower, sq, _has_src1
from concourse.dve_uop import DveOpSpec
from concourse.tile import TileContext
from concourse.tile_rust import add_dep_helper

EPS = 1e-07
SIG_OFFSET = 0.1
SIG_SCALE = 48.0

H, W = 4096, 4096
NCORES = 8
ROWS = H // NCORES          # 512 rows per core
S = 4                       # rows folded per partition
P = 128                     # partitions
NCHUNK = 8
WC = W // NCHUNK            # 512 columns per chunk
GW = WC + 2                 # chunk width incl. 1-col halo each side

F32 = mybir.dt.float32
BF16 = mybir.dt.bfloat16


# --------------------------------------------------------------------------
# custom DVE ops
# --------------------------------------------------------------------------

def _make_op(name: str, spec: Spec, row: int) -> dve_ops.DveOp:
    shas = {}
    for ver in ("v3", "v4"):
        try:
            s = DveOpSpec(name=name, opcode=row, uops=lower(spec, ver=ver),
                          rd1_en=_has_src1(spec))
            shas[ver] = s.sha(ver)
        except Exception:
            pass
    return dve_ops.DveOp(name, spec, subdim=False, uops_sha=shas)


def _register_custom_ops():
    if "ADDSQ_ANT" in dve_ops._SUB_OPCODE_FOR_NAME:
        return

    addsq_spec = Spec(
        body=Src0 + sq(Src1),
        reference=lambda in0, in1, s0, s1, imm2: (
            in0.astype(np.float32) + in1.astype(np.float32) ** 2
        ),
    )

    def _sqmulred_ref(in0, in1, c0, c1, c2):
        b = (in0.astype(np.float32) ** 2 * in1).astype(np.float32)
        acc = np.asarray(c0, np.float32).reshape(-1, 1) + b.reshape(
            b.shape[0], -1
        ).sum(axis=-1, keepdims=True)
        return b, acc

    from operator import add

    sqmulred_spec = Spec(
        body=sq(Src0) * Src1,
        accum=add,
        accum_init=C0,
        reference=_sqmulred_ref,
    )

    subsq_spec = Spec(
        body=sq(Src0 - Src1),
        reference=lambda in0, in1, s0, s1, imm2: (
            (in0.astype(np.float32) - in1.astype(np.float32)) ** 2
        ),
    )

    base = max(dve_ops._SUB_OPCODE_FOR_NAME.values()) + 1
    for i, (name, spec) in enumerate(
        [("ADDSQ_ANT", addsq_spec), ("SQMULRED_ANT", sqmulred_spec),
         ("SUBSQ_ANT", subsq_spec)]
    ):
        row = base + i
        assert row < 0x20, "custom-DVE opcode rows exhausted"
        dve_ops._SUB_OPCODE_FOR_NAME[name] = row
        op = _make_op(name, spec, row)
        dve_ops.OPS.append(op)
        dve_ops.CUSTOM_DVE_SPECS[name] = spec
        globals()["_OP_" + name] = op


_register_custom_ops()
_ADDSQ = next(o for o in dve_ops.OPS if o.name == "ADDSQ_ANT")
_SQMULRED = next(o for o in dve_ops.OPS if o.name == "SQMULRED_ANT")
_SUBSQ = next(o for o in dve_ops.OPS if o.name == "SUBSQ_ANT")


# --------------------------------------------------------------------------
# device kernel
# --------------------------------------------------------------------------

MAT_NAMES = ["I", "nI", "I0", "nI0", "I127", "nI127", "Sd", "nSu"]


def make_mats() -> np.ndarray:
    """[128, 8*128] f32 stationary matrices.

    matmul(out, lhsT, rhs): out[p, j] = sum_k lhsT[k, p] * rhs[k, j].
    Sd[k, p] = 1 iff k == p-1  (out[p] = rhs[p-1], out[0] = 0)
    Su[k, p] = 1 iff k == p+1  (out[p] = rhs[p+1], out[127] = 0)
    I0/I127 are identities with column 0 / 127 zeroed so the stencil output
    for global edge rows (row 0 -> partition 0 seg 0, row 511 -> partition
    127 seg 3) is exactly zero; the host adds those rows' contribution.
    """
    eye = np.eye(P, dtype=np.float32)
    i0 = eye.copy(); i0[:, 0] = 0.0
    i127 = eye.copy(); i127[:, 127] = 0.0
    sd = np.eye(P, k=1, dtype=np.float32)    # [k, p]: 1 at p = k+1 -> k = p-1
    su = np.eye(P, k=-1, dtype=np.float32)   # 1 at p = k-1 -> k = p+1
    import ml_dtypes
    return np.concatenate(
        [eye, -eye, i0, -i0, i127, -i127, sd, -su], axis=1
    ).astype(ml_dtypes.bfloat16).copy()


def _chunk_cols(c: int):
    """global g-grid columns [c*WC-1, c*WC+WC+1) clipped to [0, W).

    Returns (lo_clipped, n_cols, dst_off) where dst_off is the write offset
    into the GW-wide tile (pad columns at the image edge stay zero)."""
    lo = c * WC - 1
    lo_c = max(lo, 0)
    hi_c = min(c * WC + WC + 1, W)
    return lo_c, hi_c - lo_c, lo_c - lo


def build_kernel():
    nc = bacc.Bacc("TRN2", num_devices=NCORES)

    tm = nc.dram_tensor("tm", [ROWS, W], F32, kind="ExternalInput")
    li = nc.dram_tensor("li", [ROWS, W, 3], F32, kind="ExternalInput")
    mats = nc.dram_tensor("mats", [P, len(MAT_NAMES) * P], BF16,
                          kind="ExternalInput")
    out = nc.dram_tensor("out", [P, 2], F32, kind="ExternalOutput")

    tm_v = tm.ap().rearrange("(p s) w -> p s w", s=S)          # [128, 4, 4096]
    li_v = li.ap().rearrange("(p s) w c -> p s (w c)", s=S)    # [128, 4, 12288]

    GPSIMD_SUM3_CHUNKS = set(range(6))   # chunks whose channel-sum runs on Q7

    with TileContext(nc) as tc:
        with (
            tc.tile_pool(name="const", bufs=1) as cpool,
            tc.tile_pool(name="mf", bufs=1) as mfpool,
            tc.tile_pool(name="work", bufs=2) as wpool,
            tc.tile_pool(name="work1", bufs=1) as w1pool,
            tc.tile_pool(name="tmp3", bufs=3) as tmpool,
            tc.tile_pool(name="psum", bufs=1, space="PSUM") as ppool,
        ):
            mats_sb = cpool.tile([P, len(MAT_NAMES) * P], BF16)
            mats_dma = nc.sync.dma_start(out=mats_sb[:], in_=mats.ap())
            M = {n: mats_sb[:, i * P:(i + 1) * P]
                 for i, n in enumerate(MAT_NAMES)}

            acc = cpool.tile([P, 2], F32)
            nc.vector.memset(acc[:], 0.0)

            # per-partition scalar constants for activation biases
            cb = cpool.tile([P, 3], F32)
            nc.vector.memset(cb[:, 0:1], SIG_SCALE * SIG_OFFSET)
            nc.vector.memset(cb[:, 1:2], -EPS)
            nc.vector.memset(cb[:, 2:3], EPS)
            b_sig, b_neps, b_eps = cb[:, 0:1], cb[:, 1:2], cb[:, 2:3]

            # sigmoid-weight buffers, one per 4-chunk group (separate tiles
            # so group-0 sqrt/sigmoid doesn't serialize group-1 writes)
            GROUPS = [list(range(0, 4)), list(range(4, 8))]
            mfs = [mfpool.tile([P, len(g) * S * WC], F32, tag=f"mf{i}",
                               name=f"mf{i}")
                   for i, g in enumerate(GROUPS)]

            def mf_chunk(c):
                gi = 0 if c < 4 else 1
                k = c - GROUPS[gi][0]
                return mfs[gi][:, k * S * WC:(k + 1) * S * WC]

            def stencil_y(src, ps_y):
                """vertical stencil of `src` ([128, 4, GW] incl. halo col)
                into PSUM [128, 4, WC]; edge rows (p0 seg0 / p127 seg3) -> 0."""
                nc.tensor.matmul(ps_y[:, 0, :], M["Sd"], src[:, 3, 1:WC + 1],
                                 start=True, stop=False)
                nc.tensor.matmul(ps_y[:, 0, :], M["nI0"], src[:, 1, 1:WC + 1],
                                 start=False, stop=True)
                for s in (1, 2):
                    nc.tensor.matmul(ps_y[:, s, :], M["I"], src[:, s - 1, 1:WC + 1],
                                     start=True, stop=False)
                    nc.tensor.matmul(ps_y[:, s, :], M["nI"], src[:, s + 1, 1:WC + 1],
                                     start=False, stop=True)
                nc.tensor.matmul(ps_y[:, 3, :], M["I127"], src[:, 2, 1:WC + 1],
                                 start=True, stop=False)
                nc.tensor.matmul(ps_y[:, 3, :], M["nSu"], src[:, 0, 1:WC + 1],
                                 start=False, stop=True)

            def stencil_x(src, ps_x):
                """horizontal stencil (masked at edge rows) into PSUM."""
                xmat = {0: ("I0", "nI0"), 1: ("I", "nI"),
                        2: ("I", "nI"), 3: ("I127", "nI127")}
                for s in range(S):
                    a, b = xmat[s]
                    nc.tensor.matmul(ps_x[:, s, :], M[a], src[:, s, 2:GW],
                                     start=True, stop=False)
                    nc.tensor.matmul(ps_x[:, s, :], M[b], src[:, s, 0:WC],
                                     start=False, stop=True)

            # ---------------- DMA issue-order chain --------------------
            # rings drain FIFO: li 0-3, tm 0-3, li 4-7, tm 4-7
            prev_dma = [mats_dma]

            def ordered_dma(dst, srcv):
                d = nc.sync.dma_start(out=dst, in_=srcv)
                add_dep_helper(d.ins, prev_dma[0].ins, sync=False,
                               reason="dma ring order")
                prev_dma[0] = d
                return d

            li_t_all, tm_t_all = {}, {}
            g_t_all = {}

            def issue_li_dmas(cs):
                for c in cs:
                    lo_c, ncols, off = _chunk_cols(c)
                    t = wpool.tile([P, S, 3 * GW], F32, tag="li")
                    ordered_dma(t[:, :, 3 * off:3 * (off + ncols)],
                                li_v[:, :, 3 * lo_c:3 * (lo_c + ncols)])
                    li_t_all[c] = t

            def issue_tm_dmas(cs):
                for c in cs:
                    lo_c, ncols, off = _chunk_cols(c)
                    t = tmpool.tile([P, S, GW], F32, tag="tm")
                    ordered_dma(t[:, :, off:off + ncols],
                                tm_v[:, :, lo_c:lo_c + ncols])
                    tm_t_all[c] = t

            def phase_a_chunk(c):
                lo_c, ncols, off = _chunk_cols(c)
                li_t = li_t_all[c]
                g_t = wpool.tile([P, S, GW], BF16, tag="g")
                if off:
                    nc.vector.memset(g_t[:, :, 0:off], 0.0)
                if off + ncols < GW:
                    nc.vector.memset(g_t[:, :, off + ncols:GW], 0.0)
                li4 = li_t[:, :, 3 * off:3 * (off + ncols)].rearrange(
                    "p s (w c) -> p s w c", c=3)
                # channel sum: segs 0-2 on GpSimd, seg 3 on DVE reduce
                u_t = w1pool.tile([P, 3, GW], F32, tag="u")
                nc.gpsimd.tensor_add(out=u_t[:, :, off:off + ncols],
                                     in0=li4[:, 0:3, :, 0],
                                     in1=li4[:, 0:3, :, 1])
                with nc.allow_low_precision("bf16 stencil inputs"):
                    nc.gpsimd.tensor_add(out=g_t[:, 0:3, off:off + ncols],
                                         in0=u_t[:, :, off:off + ncols],
                                         in1=li4[:, 0:3, :, 2])
                    nc.vector.reduce_sum(out=g_t[:, 3:4, off:off + ncols],
                                         in_=li4[:, 3:4],
                                         axis=mybir.AxisListType.X)
                g_t_all[c] = g_t

                ps_y = ppool.tile([P, S, WC], F32, tag="psy")
                stencil_y(g_t, ps_y)

                a_t = wpool.tile([P, S, WC], F32, tag="a")
                nc.vector._custom_dve(
                    _SUBSQ, out=a_t[:],
                    in0=g_t[:, :, 2:GW], in1=g_t[:, :, 0:WC],
                )
                # m = dgx^2 + dgy^2  (1/9 scale applied inside Sqrt)
                nc.vector._custom_dve(
                    _ADDSQ,
                    out=mf_chunk(c).rearrange("p (s j) -> p s j", j=WC),
                    in0=a_t[:],
                    in1=ps_y[:],
                )

            def sqrt_sigmoid_group(gi, after=None):
                half = mfs[gi][:]
                sq_i = nc.scalar.activation(out=half, in_=half,
                                            func=mybir.ActivationFunctionType.Sqrt,
                                            scale=1.0 / 9.0)
                if after is not None:
                    add_dep_helper(sq_i.ins, after.ins, sync=False,
                                   reason="act table phase order")
                return nc.scalar.activation(
                    out=half, in_=half,
                    func=mybir.ActivationFunctionType.Sigmoid,
                    scale=-SIG_SCALE, bias=b_sig)

            def phase_b_chunk(c, sig_i, last_ln):
                lo_c, ncols, off = _chunk_cols(c)
                tm_t = tm_t_all[c]
                live = tm_t[:, :, off:off + ncols]
                nc.scalar.activation(out=live, in_=live,
                                     func=mybir.ActivationFunctionType.Relu,
                                     bias=b_neps)
                lg_t = wpool.tile([P, S, GW], BF16, tag="lg")
                ln_i = nc.scalar.activation(out=lg_t[:, :, off:off + ncols],
                                            in_=live,
                                            func=mybir.ActivationFunctionType.Ln,
                                            bias=b_eps)
                add_dep_helper(ln_i.ins, sig_i.ins, sync=False,
                               reason="act table phase order")
                if off:
                    nc.vector.memset(lg_t[:, :, 0:off], 0.0)
                if off + ncols < GW:
                    nc.vector.memset(lg_t[:, :, off + ncols:GW], 0.0)

                ps_x = ppool.tile([P, S, WC], F32, tag="psx")
                ps_y = ppool.tile([P, S, WC], F32, tag="psy")
                stencil_x(lg_t, ps_x)
                stencil_y(lg_t, ps_y)

                sig_c = mf_chunk(c)
                scr = w1pool.tile([P, S * WC], BF16, tag="scr")
                nc.vector._custom_dve(
                    _SQMULRED, out=scr[:],
                    in0=ps_x[:].rearrange("p s j -> p (s j)"), in1=sig_c,
                    s0=acc[:, 0:1], accum_out=acc[:, 0:1],
                )
                scr2 = w1pool.tile([P, S * WC], BF16, tag="scr2")
                nc.vector._custom_dve(
                    _SQMULRED, out=scr2[:],
                    in0=ps_y[:].rearrange("p s j -> p (s j)"), in1=sig_c,
                    s0=acc[:, 1:2], accum_out=acc[:, 1:2],
                )
                return ln_i

            # ---------------- schedule ---------------------------------
            issue_li_dmas(range(NCHUNK))
            issue_tm_dmas(range(NCHUNK))

            for c in GROUPS[0]:
                phase_a_chunk(c)
            sig0 = sqrt_sigmoid_group(0)
            for c in GROUPS[1]:
                phase_a_chunk(c)
            last_ln = None
            for c in GROUPS[0]:
                last_ln = phase_b_chunk(c, sig0, last_ln)
            sig1 = sqrt_sigmoid_group(1, after=last_ln)
            for c in GROUPS[1]:
                phase_b_chunk(c, sig1, last_ln)

            nc.sync.dma_start(out=out.ap(), in_=acc[:])

    nc.finalize()
    return nc


_NC_CACHE = None


def _get_nc():
    global _NC_CACHE
    if _NC_CACHE is None:
        _NC_CACHE = build_kernel()
    return _NC_CACHE


# --------------------------------------------------------------------------
# host-side edge rows (exact, float64)
# --------------------------------------------------------------------------

def _edge_contribution(tmap: np.ndarray, l_img: np.ndarray) -> float:
    """Exact contribution of global rows {512c, 512c+511} in float64."""
    rows = []
    for c in range(NCORES):
        rows.append(c * ROWS)
        rows.append(c * ROWS + ROWS - 1)

    logp = np.log(np.clip(tmap.astype(np.float64), EPS, 1.0))
    g = l_img.astype(np.float64).mean(axis=2)

    def pad_row(a, r):
        return a[r] if 0 <= r < H else np.zeros(W, np.float64)

    total = 0.0
    for r in rows:
        lc, lu, ld = logp[r], pad_row(logp, r - 1), pad_row(logp, r + 1)
        gc, gu, gd = g[r], pad_row(g, r - 1), pad_row(g, r + 1)
        zl = np.zeros(1, np.float64)

        def dx(v):
            return np.concatenate([v[1:], zl]) - np.concatenate([zl, v[:-1]])

        n = dx(lc) ** 2 + (lu - ld) ** 2
        s = np.sqrt(dx(gc) ** 2 + (gu - gd) ** 2)
        sig = 1.0 / (1.0 + np.exp(-(SIG_OFFSET - s) * SIG_SCALE))
        total += float(np.sum(n * sig))
    return total


# --------------------------------------------------------------------------
# entry point
# --------------------------------------------------------------------------

def run_device(tmap: np.ndarray, l_img: np.ndarray, **kw):
    nc = _get_nc()
    mats = make_mats()
    in_maps = [
        {
            "tm": np.ascontiguousarray(tmap[c * ROWS:(c + 1) * ROWS]),
            "li": np.ascontiguousarray(l_img[c * ROWS:(c + 1) * ROWS]),
            "mats": mats,
        }
        for c in range(NCORES)
    ]
    return bass_utils.run_bass_kernel_spmd(
        nc, in_maps, core_ids=list(range(NCORES)), **kw
    )


def kernel(tmap: np.ndarray, l_img: np.ndarray) -> np.ndarray:
    res = run_device(tmap, l_img)
    dev = sum(float(r["out"].astype(np.float64).sum()) for r in res.results)
    return np.float32(dev + _edge_contribution(tmap, l_img))


if __name__ == "__main__":
    tmap = np.random.rand(H, W).astype(np.float32)
    l_img = np.random.rand(H, W, 3).astype(np.float32)
    print(kernel(tmap, l_img))



# revision 18
# speedup vs baseline: 1.1379x; 1.0333x over previous
"""Bass/Trainium2 kernel for the edge-aware smoothness loss:

    sum over pixels of |grad log tmap|^2 * sigmoid(48*(0.1 - |grad mean(l_img)|))

Full inputs are sharded by rows across 8 NeuronCores (512 rows each).
Each core computes a partial sum over its interior rows; the 16 core-edge
rows (2 per core) are computed exactly on the host in float64 and added.

Per-core layout: partition p holds 4 consecutive rows (4p..4p+3) as 4
"segments" in the free dimension; W is processed in 8 chunks of 512 columns
(+1 halo column each side). Vertical stencil rows that cross partitions are
produced on the TensorEngine with +/-1 shift matrices; horizontal stencil and
in-partition vertical rows are also TensorEngine ident/shift matmul pairs
accumulated in PSUM. Squares/sums/reductions run on DVE custom fused ops,
transcendentals on ScalarE (table sets phased: Square -> Sqrt -> Sigmoid ->
Ln so each ACT table loads once).
"""

import sys

sys.path.insert(0, "/opt/trn_rl_repo")

import numpy as np

import concourse.bacc as bacc
import concourse.mybir as mybir
from concourse import bass_utils
from concourse import dve_ops
from concourse.dve_spec import Spec, Src0, Src1, C0, lower, sq, _has_src1
from concourse.dve_uop import DveOpSpec
from concourse.tile import TileContext
from concourse.tile_rust import add_dep_helper

EPS = 1e-07
SIG_OFFSET = 0.1
SIG_SCALE = 48.0

H, W = 4096, 4096
NCORES = 8
ROWS = H // NCORES          # 512 rows per core
S = 4                       # rows folded per partition
P = 128                     # partitions
NCHUNK = 8
WC = W // NCHUNK            # 512 columns per chunk
GW = WC + 2                 # chunk width incl. 1-col halo each side

F32 = mybir.dt.float32
BF16 = mybir.dt.bfloat16


# --------------------------------------------------------------------------
# custom DVE ops
# --------------------------------------------------------------------------

def _make_op(name: str, spec: Spec, row: int) -> dve_ops.DveOp:
    shas = {}
    for ver in ("v3", "v4"):
        try:
            s = DveOpSpec(name=name, opcode=row, uops=lower(spec, ver=ver),
                          rd1_en=_has_src1(spec))
            shas[ver] = s.sha(ver)
        except Exception:
            pass
    return dve_ops.DveOp(name, spec, subdim=False, uops_sha=shas)


def _register_custom_ops():
    if "ADDSQ_ANT" in dve_ops._SUB_OPCODE_FOR_NAME:
        return

    addsq_spec = Spec(
        body=Src0 + sq(Src1),
        reference=lambda in0, in1, s0, s1, imm2: (
            in0.astype(np.float32) + in1.astype(np.float32) ** 2
        ),
    )

    def _sqmulred_ref(in0, in1, c0, c1, c2):
        b = (in0.astype(np.float32) ** 2 * in1).astype(np.float32)
        acc = np.asarray(c0, np.float32).reshape(-1, 1) + b.reshape(
            b.shape[0], -1
        ).sum(axis=-1, keepdims=True)
        return b, acc

    from operator import add

    sqmulred_spec = Spec(
        body=sq(Src0) * Src1,
        accum=add,
        accum_init=C0,
        reference=_sqmulred_ref,
    )

    subsq_spec = Spec(
        body=sq(Src0 - Src1),
        reference=lambda in0, in1, s0, s1, imm2: (
            (in0.astype(np.float32) - in1.astype(np.float32)) ** 2
        ),
    )

    base = max(dve_ops._SUB_OPCODE_FOR_NAME.values()) + 1
    for i, (name, spec) in enumerate(
        [("ADDSQ_ANT", addsq_spec), ("SQMULRED_ANT", sqmulred_spec),
         ("SUBSQ_ANT", subsq_spec)]
    ):
        row = base + i
        assert row < 0x20, "custom-DVE opcode rows exhausted"
        dve_ops._SUB_OPCODE_FOR_NAME[name] = row
        op = _make_op(name, spec, row)
        dve_ops.OPS.append(op)
        dve_ops.CUSTOM_DVE_SPECS[name] = spec
        globals()["_OP_" + name] = op


_register_custom_ops()
_ADDSQ = next(o for o in dve_ops.OPS if o.name == "ADDSQ_ANT")
_SQMULRED = next(o for o in dve_ops.OPS if o.name == "SQMULRED_ANT")
_SUBSQ = next(o for o in dve_ops.OPS if o.name == "SUBSQ_ANT")


# --------------------------------------------------------------------------
# device kernel
# --------------------------------------------------------------------------

MAT_NAMES = ["I", "nI", "I0", "nI0", "I127", "nI127", "Sd", "nSu"]


def make_mats() -> np.ndarray:
    """[128, 8*128] f32 stationary matrices.

    matmul(out, lhsT, rhs): out[p, j] = sum_k lhsT[k, p] * rhs[k, j].
    Sd[k, p] = 1 iff k == p-1  (out[p] = rhs[p-1], out[0] = 0)
    Su[k, p] = 1 iff k == p+1  (out[p] = rhs[p+1], out[127] = 0)
    I0/I127 are identities with column 0 / 127 zeroed so the stencil output
    for global edge rows (row 0 -> partition 0 seg 0, row 511 -> partition
    127 seg 3) is exactly zero; the host adds those rows' contribution.
    """
    eye = np.eye(P, dtype=np.float32)
    i0 = eye.copy(); i0[:, 0] = 0.0
    i127 = eye.copy(); i127[:, 127] = 0.0
    sd = np.eye(P, k=1, dtype=np.float32)    # [k, p]: 1 at p = k+1 -> k = p-1
    su = np.eye(P, k=-1, dtype=np.float32)   # 1 at p = k-1 -> k = p+1
    import ml_dtypes
    return np.concatenate(
        [eye, -eye, i0, -i0, i127, -i127, sd, -su], axis=1
    ).astype(ml_dtypes.bfloat16).copy()


def _chunk_cols(c: int):
    """global g-grid columns [c*WC-1, c*WC+WC+1) clipped to [0, W).

    Returns (lo_clipped, n_cols, dst_off) where dst_off is the write offset
    into the GW-wide tile (pad columns at the image edge stay zero)."""
    lo = c * WC - 1
    lo_c = max(lo, 0)
    hi_c = min(c * WC + WC + 1, W)
    return lo_c, hi_c - lo_c, lo_c - lo


def build_kernel():
    nc = bacc.Bacc("TRN2", num_devices=NCORES)

    tm = nc.dram_tensor("tm", [ROWS, W], F32, kind="ExternalInput")
    li = nc.dram_tensor("li", [ROWS, W, 3], F32, kind="ExternalInput")
    mats = nc.dram_tensor("mats", [P, len(MAT_NAMES) * P], BF16,
                          kind="ExternalInput")
    out = nc.dram_tensor("out", [P, 2], F32, kind="ExternalOutput")

    tm_v = tm.ap().rearrange("(p s) w -> p s w", s=S)          # [128, 4, 4096]
    li_v = li.ap().rearrange("(p s) w c -> p s (w c)", s=S)    # [128, 4, 12288]

    GPSIMD_SUM3_CHUNKS = set(range(6))   # chunks whose channel-sum runs on Q7

    with TileContext(nc) as tc:
        with (
            tc.tile_pool(name="const", bufs=1) as cpool,
            tc.tile_pool(name="mf", bufs=1) as mfpool,
            tc.tile_pool(name="work", bufs=2) as wpool,
            tc.tile_pool(name="work1", bufs=1) as w1pool,
            tc.tile_pool(name="tmp3", bufs=3) as tmpool,
            tc.tile_pool(name="psum", bufs=1, space="PSUM") as ppool,
        ):
            mats_sb = cpool.tile([P, len(MAT_NAMES) * P], BF16)
            mats_dma = nc.sync.dma_start(out=mats_sb[:], in_=mats.ap())
            M = {n: mats_sb[:, i * P:(i + 1) * P]
                 for i, n in enumerate(MAT_NAMES)}

            acc = cpool.tile([P, 2], F32)
            nc.vector.memset(acc[:], 0.0)

            # per-partition scalar constants for activation biases
            cb = cpool.tile([P, 3], F32)
            nc.vector.memset(cb[:, 0:1], SIG_SCALE * SIG_OFFSET)
            nc.vector.memset(cb[:, 1:2], -EPS)
            nc.vector.memset(cb[:, 2:3], EPS)
            b_sig, b_neps, b_eps = cb[:, 0:1], cb[:, 1:2], cb[:, 2:3]

            # sigmoid-weight buffers, one per 4-chunk group (separate tiles
            # so group-0 sqrt/sigmoid doesn't serialize group-1 writes)
            GROUPS = [list(range(0, 4)), list(range(4, 8))]
            mfs = [mfpool.tile([P, len(g) * S * WC], F32, tag=f"mf{i}",
                               name=f"mf{i}")
                   for i, g in enumerate(GROUPS)]

            def mf_chunk(c):
                gi = 0 if c < 4 else 1
                k = c - GROUPS[gi][0]
                return mfs[gi][:, k * S * WC:(k + 1) * S * WC]

            def stencil_y(src, ps_y):
                """vertical stencil of `src` ([128, 4, GW] incl. halo col)
                into PSUM [128, 4, WC]; edge rows (p0 seg0 / p127 seg3) -> 0."""
                nc.tensor.matmul(ps_y[:, 0, :], M["Sd"], src[:, 3, 1:WC + 1],
                                 start=True, stop=False)
                nc.tensor.matmul(ps_y[:, 0, :], M["nI0"], src[:, 1, 1:WC + 1],
                                 start=False, stop=True)
                for s in (1, 2):
                    nc.tensor.matmul(ps_y[:, s, :], M["I"], src[:, s - 1, 1:WC + 1],
                                     start=True, stop=False)
                    nc.tensor.matmul(ps_y[:, s, :], M["nI"], src[:, s + 1, 1:WC + 1],
                                     start=False, stop=True)
                nc.tensor.matmul(ps_y[:, 3, :], M["I127"], src[:, 2, 1:WC + 1],
                                 start=True, stop=False)
                nc.tensor.matmul(ps_y[:, 3, :], M["nSu"], src[:, 0, 1:WC + 1],
                                 start=False, stop=True)

            def stencil_x(src, ps_x):
                """horizontal stencil (masked at edge rows) into PSUM."""
                xmat = {0: ("I0", "nI0"), 1: ("I", "nI"),
                        2: ("I", "nI"), 3: ("I127", "nI127")}
                for s in range(S):
                    a, b = xmat[s]
                    nc.tensor.matmul(ps_x[:, s, :], M[a], src[:, s, 2:GW],
                                     start=True, stop=False)
                    nc.tensor.matmul(ps_x[:, s, :], M[b], src[:, s, 0:WC],
                                     start=False, stop=True)

            # ---------------- DMA issue-order chain --------------------
            # rings drain FIFO: li 0-3, tm 0-3, li 4-7, tm 4-7
            prev_dma = [mats_dma]

            def ordered_dma(dst, srcv):
                d = nc.sync.dma_start(out=dst, in_=srcv)
                add_dep_helper(d.ins, prev_dma[0].ins, sync=False,
                               reason="dma ring order")
                prev_dma[0] = d
                return d

            li_t_all, tm_t_all = {}, {}
            g_t_all = {}

            def issue_li_dmas(cs):
                for c in cs:
                    lo_c, ncols, off = _chunk_cols(c)
                    t = wpool.tile([P, S, 3 * GW], F32, tag="li")
                    ordered_dma(t[:, :, 3 * off:3 * (off + ncols)],
                                li_v[:, :, 3 * lo_c:3 * (lo_c + ncols)])
                    li_t_all[c] = t

            def issue_tm_dmas(cs):
                for c in cs:
                    lo_c, ncols, off = _chunk_cols(c)
                    t = tmpool.tile([P, S, GW], F32, tag="tm")
                    ordered_dma(t[:, :, off:off + ncols],
                                tm_v[:, :, lo_c:lo_c + ncols])
                    tm_t_all[c] = t

            def phase_a_chunk(c):
                lo_c, ncols, off = _chunk_cols(c)
                li_t = li_t_all[c]
                g_t = wpool.tile([P, S, GW], BF16, tag="g")
                if off:
                    nc.vector.memset(g_t[:, :, 0:off], 0.0)
                if off + ncols < GW:
                    nc.vector.memset(g_t[:, :, off + ncols:GW], 0.0)
                li4 = li_t[:, :, 3 * off:3 * (off + ncols)].rearrange(
                    "p s (w c) -> p s w c", c=3)
                # channel sum: segs 0-2 on GpSimd, seg 3 on DVE reduce
                u_t = w1pool.tile([P, 3, GW], F32, tag="u")
                nc.gpsimd.tensor_add(out=u_t[:, :, off:off + ncols],
                                     in0=li4[:, 0:3, :, 0],
                                     in1=li4[:, 0:3, :, 1])
                with nc.allow_low_precision("bf16 stencil inputs"):
                    nc.gpsimd.tensor_add(out=g_t[:, 0:3, off:off + ncols],
                                         in0=u_t[:, :, off:off + ncols],
                                         in1=li4[:, 0:3, :, 2])
                    nc.vector.reduce_sum(out=g_t[:, 3:4, off:off + ncols],
                                         in_=li4[:, 3:4],
                                         axis=mybir.AxisListType.X)
                g_t_all[c] = g_t

                ps_y = ppool.tile([P, S, WC], F32, tag="psy")
                stencil_y(g_t, ps_y)

                a_t = wpool.tile([P, S, WC], F32, tag="a")
                nc.vector._custom_dve(
                    _SUBSQ, out=a_t[:],
                    in0=g_t[:, :, 2:GW], in1=g_t[:, :, 0:WC],
                )
                # m = dgx^2 + dgy^2  (1/9 scale applied inside Sqrt)
                nc.vector._custom_dve(
                    _ADDSQ,
                    out=mf_chunk(c).rearrange("p (s j) -> p s j", j=WC),
                    in0=a_t[:],
                    in1=ps_y[:],
                )

            def sqrt_sigmoid_group(gi, after=None):
                half = mfs[gi][:]
                sq_i = nc.scalar.activation(out=half, in_=half,
                                            func=mybir.ActivationFunctionType.Sqrt,
                                            scale=1.0 / 9.0)
                if after is not None:
                    add_dep_helper(sq_i.ins, after.ins, sync=False,
                                   reason="act table phase order")
                return nc.scalar.activation(
                    out=half, in_=half,
                    func=mybir.ActivationFunctionType.Sigmoid,
                    scale=-SIG_SCALE, bias=b_sig)

            def phase_b_chunk(c, sig_i, last_ln):
                lo_c, ncols, off = _chunk_cols(c)
                tm_t = tm_t_all[c]
                live = tm_t[:, :, off:off + ncols]
                lg_t = wpool.tile([P, S, GW], BF16, tag="lg")
                ln_i = nc.scalar.activation(out=lg_t[:, :, off:off + ncols],
                                            in_=live,
                                            func=mybir.ActivationFunctionType.Ln,
                                            bias=b_eps)
                add_dep_helper(ln_i.ins, sig_i.ins, sync=False,
                               reason="act table phase order")
                if off:
                    nc.vector.memset(lg_t[:, :, 0:off], 0.0)
                if off + ncols < GW:
                    nc.vector.memset(lg_t[:, :, off + ncols:GW], 0.0)

                ps_x = ppool.tile([P, S, WC], F32, tag="psx")
                ps_y = ppool.tile([P, S, WC], F32, tag="psy")
                stencil_x(lg_t, ps_x)
                stencil_y(lg_t, ps_y)

                sig_c = mf_chunk(c)
                scr = w1pool.tile([P, S * WC], BF16, tag="scr")
                nc.vector._custom_dve(
                    _SQMULRED, out=scr[:],
                    in0=ps_x[:].rearrange("p s j -> p (s j)"), in1=sig_c,
                    s0=acc[:, 0:1], accum_out=acc[:, 0:1],
                )
                scr2 = w1pool.tile([P, S * WC], BF16, tag="scr2")
                nc.vector._custom_dve(
                    _SQMULRED, out=scr2[:],
                    in0=ps_y[:].rearrange("p s j -> p (s j)"), in1=sig_c,
                    s0=acc[:, 1:2], accum_out=acc[:, 1:2],
                )
                return ln_i

            # ---------------- schedule ---------------------------------
            issue_li_dmas(range(NCHUNK))
            issue_tm_dmas(range(NCHUNK))

            for c in GROUPS[0]:
                phase_a_chunk(c)
            sig0 = sqrt_sigmoid_group(0)
            for c in GROUPS[1]:
                phase_a_chunk(c)
            last_ln = None
            for c in GROUPS[0]:
                last_ln = phase_b_chunk(c, sig0, last_ln)
            sig1 = sqrt_sigmoid_group(1, after=last_ln)
            for c in GROUPS[1]:
                phase_b_chunk(c, sig1, last_ln)

            nc.sync.dma_start(out=out.ap(), in_=acc[:])

    nc.finalize()
    return nc


_NC_CACHE = None


def _get_nc():
    global _NC_CACHE
    if _NC_CACHE is None:
        _NC_CACHE = build_kernel()
    return _NC_CACHE


# --------------------------------------------------------------------------
# host-side edge rows (exact, float64)
# --------------------------------------------------------------------------

def _edge_contribution(tmap: np.ndarray, l_img: np.ndarray) -> float:
    """Exact contribution of global rows {512c, 512c+511} in float64."""
    rows = []
    for c in range(NCORES):
        rows.append(c * ROWS)
        rows.append(c * ROWS + ROWS - 1)

    logp = np.log(np.clip(tmap.astype(np.float64), EPS, 1.0))
    g = l_img.astype(np.float64).mean(axis=2)

    def pad_row(a, r):
        return a[r] if 0 <= r < H else np.zeros(W, np.float64)

    total = 0.0
    for r in rows:
        lc, lu, ld = logp[r], pad_row(logp, r - 1), pad_row(logp, r + 1)
        gc, gu, gd = g[r], pad_row(g, r - 1), pad_row(g, r + 1)
        zl = np.zeros(1, np.float64)

        def dx(v):
            return np.concatenate([v[1:], zl]) - np.concatenate([zl, v[:-1]])

        n = dx(lc) ** 2 + (lu - ld) ** 2
        s = np.sqrt(dx(gc) ** 2 + (gu - gd) ** 2)
        sig = 1.0 / (1.0 + np.exp(-(SIG_OFFSET - s) * SIG_SCALE))
        total += float(np.sum(n * sig))
    return total


# --------------------------------------------------------------------------
# entry point
# --------------------------------------------------------------------------

def run_device(tmap: np.ndarray, l_img: np.ndarray, **kw):
    nc = _get_nc()
    mats = make_mats()
    in_maps = [
        {
            "tm": np.ascontiguousarray(tmap[c * ROWS:(c + 1) * ROWS]),
            "li": np.ascontiguousarray(l_img[c * ROWS:(c + 1) * ROWS]),
            "mats": mats,
        }
        for c in range(NCORES)
    ]
    return bass_utils.run_bass_kernel_spmd(
        nc, in_maps, core_ids=list(range(NCORES)), **kw
    )


def kernel(tmap: np.ndarray, l_img: np.ndarray) -> np.ndarray:
    res = run_device(tmap, l_img)
    dev = sum(float(r["out"].astype(np.float64).sum()) for r in res.results)
    return np.float32(dev + _edge_contribution(tmap, l_img))


if __name__ == "__main__":
    tmap = np.random.rand(H, W).astype(np.float32)
    l_img = np.random.rand(H, W, 3).astype(np.float32)
    print(kernel(tmap, l_img))



# revision 20
# speedup vs baseline: 1.1697x; 1.0279x over previous
"""Bass/Trainium2 kernel for the edge-aware smoothness loss:

    sum over pixels of |grad log tmap|^2 * sigmoid(48*(0.1 - |grad mean(l_img)|))

Full inputs are sharded by rows across 8 NeuronCores (512 rows each).
Each core computes a partial sum over its interior rows; the 16 core-edge
rows (2 per core) are computed exactly on the host in float64 and added.

Per-core layout: partition p holds 4 consecutive rows (4p..4p+3) as 4
"segments" in the free dimension; W is processed in 8 chunks of 512 columns
(+1 halo column each side). Vertical stencil rows that cross partitions are
produced on the TensorEngine with +/-1 shift matrices; horizontal stencil and
in-partition vertical rows are also TensorEngine ident/shift matmul pairs
accumulated in PSUM. Squares/sums/reductions run on DVE custom fused ops,
transcendentals on ScalarE (table sets phased: Square -> Sqrt -> Sigmoid ->
Ln so each ACT table loads once).
"""

import sys

sys.path.insert(0, "/opt/trn_rl_repo")

import numpy as np

import concourse.bacc as bacc
import concourse.mybir as mybir
from concourse import bass_utils
from concourse import dve_ops
from concourse.dve_spec import Spec, Src0, Src1, C0, lower, sq, _has_src1
from concourse.dve_uop import DveOpSpec
from concourse.tile import TileContext
from concourse.tile_rust import add_dep_helper

EPS = 1e-07
SIG_OFFSET = 0.1
SIG_SCALE = 48.0

H, W = 4096, 4096
NCORES = 8
ROWS = H // NCORES          # 512 rows per core
S = 4                       # rows folded per partition
P = 128                     # partitions
NCHUNK = 8
WC = W // NCHUNK            # 512 columns per chunk
GW = WC + 2                 # chunk width incl. 1-col halo each side

F32 = mybir.dt.float32
BF16 = mybir.dt.bfloat16


# --------------------------------------------------------------------------
# custom DVE ops
# --------------------------------------------------------------------------

def _make_op(name: str, spec: Spec, row: int) -> dve_ops.DveOp:
    shas = {}
    for ver in ("v3", "v4"):
        try:
            s = DveOpSpec(name=name, opcode=row, uops=lower(spec, ver=ver),
                          rd1_en=_has_src1(spec))
            shas[ver] = s.sha(ver)
        except Exception:
            pass
    return dve_ops.DveOp(name, spec, subdim=False, uops_sha=shas)


def _register_custom_ops():
    if "ADDSQ_ANT" in dve_ops._SUB_OPCODE_FOR_NAME:
        return

    addsq_spec = Spec(
        body=Src0 + sq(Src1),
        reference=lambda in0, in1, s0, s1, imm2: (
            in0.astype(np.float32) + in1.astype(np.float32) ** 2
        ),
    )

    def _sqmulred_ref(in0, in1, c0, c1, c2):
        b = (in0.astype(np.float32) ** 2 * in1).astype(np.float32)
        acc = np.asarray(c0, np.float32).reshape(-1, 1) + b.reshape(
            b.shape[0], -1
        ).sum(axis=-1, keepdims=True)
        return b, acc

    from operator import add

    sqmulred_spec = Spec(
        body=sq(Src0) * Src1,
        accum=add,
        accum_init=C0,
        reference=_sqmulred_ref,
    )

    subsq_spec = Spec(
        body=sq(Src0 - Src1),
        reference=lambda in0, in1, s0, s1, imm2: (
            (in0.astype(np.float32) - in1.astype(np.float32)) ** 2
        ),
    )

    base = max(dve_ops._SUB_OPCODE_FOR_NAME.values()) + 1
    for i, (name, spec) in enumerate(
        [("ADDSQ_ANT", addsq_spec), ("SQMULRED_ANT", sqmulred_spec),
         ("SUBSQ_ANT", subsq_spec)]
    ):
        row = base + i
        assert row < 0x20, "custom-DVE opcode rows exhausted"
        dve_ops._SUB_OPCODE_FOR_NAME[name] = row
        op = _make_op(name, spec, row)
        dve_ops.OPS.append(op)
        dve_ops.CUSTOM_DVE_SPECS[name] = spec
        globals()["_OP_" + name] = op


_register_custom_ops()
_ADDSQ = next(o for o in dve_ops.OPS if o.name == "ADDSQ_ANT")
_SQMULRED = next(o for o in dve_ops.OPS if o.name == "SQMULRED_ANT")
_SUBSQ = next(o for o in dve_ops.OPS if o.name == "SUBSQ_ANT")


# --------------------------------------------------------------------------
# device kernel
# --------------------------------------------------------------------------

MAT_NAMES = ["I", "nI", "I0", "nI0", "I127", "nI127", "Sd", "nSu"]


def make_mats() -> np.ndarray:
    """[128, 8*128] f32 stationary matrices.

    matmul(out, lhsT, rhs): out[p, j] = sum_k lhsT[k, p] * rhs[k, j].
    Sd[k, p] = 1 iff k == p-1  (out[p] = rhs[p-1], out[0] = 0)
    Su[k, p] = 1 iff k == p+1  (out[p] = rhs[p+1], out[127] = 0)
    I0/I127 are identities with column 0 / 127 zeroed so the stencil output
    for global edge rows (row 0 -> partition 0 seg 0, row 511 -> partition
    127 seg 3) is exactly zero; the host adds those rows' contribution.
    """
    eye = np.eye(P, dtype=np.float32)
    i0 = eye.copy(); i0[:, 0] = 0.0
    i127 = eye.copy(); i127[:, 127] = 0.0
    sd = np.eye(P, k=1, dtype=np.float32)    # [k, p]: 1 at p = k+1 -> k = p-1
    su = np.eye(P, k=-1, dtype=np.float32)   # 1 at p = k-1 -> k = p+1
    import ml_dtypes
    return np.concatenate(
        [eye, -eye, i0, -i0, i127, -i127, sd, -su], axis=1
    ).astype(ml_dtypes.bfloat16).copy()


def _chunk_cols(c: int):
    """global g-grid columns [c*WC-1, c*WC+WC+1) clipped to [0, W).

    Returns (lo_clipped, n_cols, dst_off) where dst_off is the write offset
    into the GW-wide tile (pad columns at the image edge stay zero)."""
    lo = c * WC - 1
    lo_c = max(lo, 0)
    hi_c = min(c * WC + WC + 1, W)
    return lo_c, hi_c - lo_c, lo_c - lo


def build_kernel():
    nc = bacc.Bacc("TRN2", num_devices=NCORES)

    tm = nc.dram_tensor("tm", [ROWS, W], F32, kind="ExternalInput")
    li = nc.dram_tensor("li", [ROWS, W, 3], F32, kind="ExternalInput")
    mats = nc.dram_tensor("mats", [P, len(MAT_NAMES) * P], BF16,
                          kind="ExternalInput")
    out = nc.dram_tensor("out", [P, 2], F32, kind="ExternalOutput")

    tm_v = tm.ap().rearrange("(p s) w -> p s w", s=S)          # [128, 4, 4096]
    li_v = li.ap().rearrange("(p s) w c -> p s (w c)", s=S)    # [128, 4, 12288]

    GPSIMD_SUM3_CHUNKS = set(range(6))   # chunks whose channel-sum runs on Q7

    with TileContext(nc) as tc:
        with (
            tc.tile_pool(name="const", bufs=1) as cpool,
            tc.tile_pool(name="mf", bufs=1) as mfpool,
            tc.tile_pool(name="work", bufs=2) as wpool,
            tc.tile_pool(name="work1", bufs=1) as w1pool,
            tc.tile_pool(name="tmp3", bufs=3) as tmpool,
            tc.tile_pool(name="psum", bufs=1, space="PSUM") as ppool,
        ):
            mats_sb = cpool.tile([P, len(MAT_NAMES) * P], BF16)
            mats_dma = nc.sync.dma_start(out=mats_sb[:], in_=mats.ap())
            M = {n: mats_sb[:, i * P:(i + 1) * P]
                 for i, n in enumerate(MAT_NAMES)}

            acc = cpool.tile([P, 2], F32)
            nc.vector.memset(acc[:], 0.0)

            # per-partition scalar constants for activation biases
            cb = cpool.tile([P, 3], F32)
            nc.vector.memset(cb[:, 0:1], SIG_SCALE * SIG_OFFSET)
            nc.vector.memset(cb[:, 1:2], -EPS)
            nc.vector.memset(cb[:, 2:3], EPS)
            b_sig, b_neps, b_eps = cb[:, 0:1], cb[:, 1:2], cb[:, 2:3]

            # sigmoid-weight buffers, one per group (separate tiles so one
            # group's sqrt/sigmoid doesn't serialize another's writes).
            # The trailing groups are split small: their sqrt+sigmoid sit on
            # the critical tail behind the last li chunks, so a half-size
            # barrier lets phase_b 4-5 start while A6/A7 are still landing.
            GROUPS = [[0, 1, 2, 3], [4, 5], [6, 7]]
            mfs = [mfpool.tile([P, len(g) * S * WC], F32, tag=f"mf{i}",
                               name=f"mf{i}")
                   for i, g in enumerate(GROUPS)]

            def mf_chunk(c):
                gi = next(i for i, g in enumerate(GROUPS) if c in g)
                k = c - GROUPS[gi][0]
                return mfs[gi][:, k * S * WC:(k + 1) * S * WC]

            def stencil_y(src, ps_y):
                """vertical stencil of `src` ([128, 4, GW] incl. halo col)
                into PSUM [128, 4, WC]; edge rows (p0 seg0 / p127 seg3) -> 0."""
                nc.tensor.matmul(ps_y[:, 0, :], M["Sd"], src[:, 3, 1:WC + 1],
                                 start=True, stop=False)
                nc.tensor.matmul(ps_y[:, 0, :], M["nI0"], src[:, 1, 1:WC + 1],
                                 start=False, stop=True)
                for s in (1, 2):
                    nc.tensor.matmul(ps_y[:, s, :], M["I"], src[:, s - 1, 1:WC + 1],
                                     start=True, stop=False)
                    nc.tensor.matmul(ps_y[:, s, :], M["nI"], src[:, s + 1, 1:WC + 1],
                                     start=False, stop=True)
                nc.tensor.matmul(ps_y[:, 3, :], M["I127"], src[:, 2, 1:WC + 1],
                                 start=True, stop=False)
                nc.tensor.matmul(ps_y[:, 3, :], M["nSu"], src[:, 0, 1:WC + 1],
                                 start=False, stop=True)

            def stencil_x(src, ps_x):
                """horizontal stencil (masked at edge rows) into PSUM."""
                xmat = {0: ("I0", "nI0"), 1: ("I", "nI"),
                        2: ("I", "nI"), 3: ("I127", "nI127")}
                for s in range(S):
                    a, b = xmat[s]
                    nc.tensor.matmul(ps_x[:, s, :], M[a], src[:, s, 2:GW],
                                     start=True, stop=False)
                    nc.tensor.matmul(ps_x[:, s, :], M[b], src[:, s, 0:WC],
                                     start=False, stop=True)

            # ---------------- DMA issue-order chain --------------------
            # rings drain FIFO: li 0-3, tm 0-3, li 4-7, tm 4-7
            prev_dma = [mats_dma]

            def ordered_dma(dst, srcv):
                d = nc.sync.dma_start(out=dst, in_=srcv)
                add_dep_helper(d.ins, prev_dma[0].ins, sync=False,
                               reason="dma ring order")
                prev_dma[0] = d
                return d

            li_t_all, tm_t_all = {}, {}
            g_t_all = {}

            def issue_li_dmas(cs):
                for c in cs:
                    lo_c, ncols, off = _chunk_cols(c)
                    t = wpool.tile([P, S, 3 * GW], F32, tag="li")
                    ordered_dma(t[:, :, 3 * off:3 * (off + ncols)],
                                li_v[:, :, 3 * lo_c:3 * (lo_c + ncols)])
                    li_t_all[c] = t

            def issue_tm_dmas(cs):
                for c in cs:
                    lo_c, ncols, off = _chunk_cols(c)
                    t = tmpool.tile([P, S, GW], F32, tag="tm")
                    ordered_dma(t[:, :, off:off + ncols],
                                tm_v[:, :, lo_c:lo_c + ncols])
                    tm_t_all[c] = t

            def phase_a_chunk(c):
                lo_c, ncols, off = _chunk_cols(c)
                li_t = li_t_all[c]
                g_t = wpool.tile([P, S, GW], BF16, tag="g")
                if off:
                    nc.vector.memset(g_t[:, :, 0:off], 0.0)
                if off + ncols < GW:
                    nc.vector.memset(g_t[:, :, off + ncols:GW], 0.0)
                li4 = li_t[:, :, 3 * off:3 * (off + ncols)].rearrange(
                    "p s (w c) -> p s w c", c=3)
                # channel sum: segs 0-2 on GpSimd, seg 3 on DVE reduce
                u_t = w1pool.tile([P, 3, GW], F32, tag="u")
                nc.gpsimd.tensor_add(out=u_t[:, :, off:off + ncols],
                                     in0=li4[:, 0:3, :, 0],
                                     in1=li4[:, 0:3, :, 1])
                with nc.allow_low_precision("bf16 stencil inputs"):
                    nc.gpsimd.tensor_add(out=g_t[:, 0:3, off:off + ncols],
                                         in0=u_t[:, :, off:off + ncols],
                                         in1=li4[:, 0:3, :, 2])
                    nc.vector.reduce_sum(out=g_t[:, 3:4, off:off + ncols],
                                         in_=li4[:, 3:4],
                                         axis=mybir.AxisListType.X)
                g_t_all[c] = g_t

                ps_y = ppool.tile([P, S, WC], F32, tag="psy")
                stencil_y(g_t, ps_y)

                a_t = wpool.tile([P, S, WC], F32, tag="a")
                nc.vector._custom_dve(
                    _SUBSQ, out=a_t[:],
                    in0=g_t[:, :, 2:GW], in1=g_t[:, :, 0:WC],
                )
                # m = dgx^2 + dgy^2  (1/9 scale applied inside Sqrt)
                nc.vector._custom_dve(
                    _ADDSQ,
                    out=mf_chunk(c).rearrange("p (s j) -> p s j", j=WC),
                    in0=a_t[:],
                    in1=ps_y[:],
                )

            def sqrt_sigmoid_group(gi, after=None):
                half = mfs[gi][:]
                sq_i = nc.scalar.activation(out=half, in_=half,
                                            func=mybir.ActivationFunctionType.Sqrt,
                                            scale=1.0 / 9.0)
                if after is not None:
                    add_dep_helper(sq_i.ins, after.ins, sync=False,
                                   reason="act table phase order")
                return nc.scalar.activation(
                    out=half, in_=half,
                    func=mybir.ActivationFunctionType.Sigmoid,
                    scale=-SIG_SCALE, bias=b_sig)

            def phase_b_chunk(c, sig_i, last_ln):
                lo_c, ncols, off = _chunk_cols(c)
                tm_t = tm_t_all[c]
                live = tm_t[:, :, off:off + ncols]
                lg_t = wpool.tile([P, S, GW], BF16, tag="lg")
                ln_i = nc.scalar.activation(out=lg_t[:, :, off:off + ncols],
                                            in_=live,
                                            func=mybir.ActivationFunctionType.Ln,
                                            bias=b_eps)
                add_dep_helper(ln_i.ins, sig_i.ins, sync=False,
                               reason="act table phase order")
                if off:
                    nc.vector.memset(lg_t[:, :, 0:off], 0.0)
                if off + ncols < GW:
                    nc.vector.memset(lg_t[:, :, off + ncols:GW], 0.0)

                ps_x = ppool.tile([P, S, WC], F32, tag="psx")
                ps_y = ppool.tile([P, S, WC], F32, tag="psy")
                stencil_x(lg_t, ps_x)
                stencil_y(lg_t, ps_y)

                sig_c = mf_chunk(c)
                scr = w1pool.tile([P, S * WC], BF16, tag="scr")
                nc.vector._custom_dve(
                    _SQMULRED, out=scr[:],
                    in0=ps_x[:].rearrange("p s j -> p (s j)"), in1=sig_c,
                    s0=acc[:, 0:1], accum_out=acc[:, 0:1],
                )
                scr2 = w1pool.tile([P, S * WC], BF16, tag="scr2")
                nc.vector._custom_dve(
                    _SQMULRED, out=scr2[:],
                    in0=ps_y[:].rearrange("p s j -> p (s j)"), in1=sig_c,
                    s0=acc[:, 1:2], accum_out=acc[:, 1:2],
                )
                return ln_i

            # ---------------- schedule ---------------------------------
            issue_li_dmas(range(NCHUNK))
            issue_tm_dmas(range(NCHUNK))

            for c in GROUPS[0]:
                phase_a_chunk(c)
            sig0 = sqrt_sigmoid_group(0)
            for c in GROUPS[1] + GROUPS[2]:
                phase_a_chunk(c)
            last_ln = None
            for c in GROUPS[0]:
                last_ln = phase_b_chunk(c, sig0, last_ln)
            sig1 = sqrt_sigmoid_group(1, after=last_ln)
            for c in GROUPS[1]:
                last_ln = phase_b_chunk(c, sig1, last_ln)
            sig2 = sqrt_sigmoid_group(2, after=last_ln)
            for c in GROUPS[2]:
                phase_b_chunk(c, sig2, last_ln)

            nc.sync.dma_start(out=out.ap(), in_=acc[:])

    nc.finalize()
    return nc


_NC_CACHE = None


def _get_nc():
    global _NC_CACHE
    if _NC_CACHE is None:
        _NC_CACHE = build_kernel()
    return _NC_CACHE


# --------------------------------------------------------------------------
# host-side edge rows (exact, float64)
# --------------------------------------------------------------------------

def _edge_contribution(tmap: np.ndarray, l_img: np.ndarray) -> float:
    """Exact contribution of global rows {512c, 512c+511} in float64."""
    rows = []
    for c in range(NCORES):
        rows.append(c * ROWS)
        rows.append(c * ROWS + ROWS - 1)

    logp = np.log(np.clip(tmap.astype(np.float64), EPS, 1.0))
    g = l_img.astype(np.float64).mean(axis=2)

    def pad_row(a, r):
        return a[r] if 0 <= r < H else np.zeros(W, np.float64)

    total = 0.0
    for r in rows:
        lc, lu, ld = logp[r], pad_row(logp, r - 1), pad_row(logp, r + 1)
        gc, gu, gd = g[r], pad_row(g, r - 1), pad_row(g, r + 1)
        zl = np.zeros(1, np.float64)

        def dx(v):
            return np.concatenate([v[1:], zl]) - np.concatenate([zl, v[:-1]])

        n = dx(lc) ** 2 + (lu - ld) ** 2
        s = np.sqrt(dx(gc) ** 2 + (gu - gd) ** 2)
        sig = 1.0 / (1.0 + np.exp(-(SIG_OFFSET - s) * SIG_SCALE))
        total += float(np.sum(n * sig))
    return total


# --------------------------------------------------------------------------
# entry point
# --------------------------------------------------------------------------

def run_device(tmap: np.ndarray, l_img: np.ndarray, **kw):
    nc = _get_nc()
    mats = make_mats()
    in_maps = [
        {
            "tm": np.ascontiguousarray(tmap[c * ROWS:(c + 1) * ROWS]),
            "li": np.ascontiguousarray(l_img[c * ROWS:(c + 1) * ROWS]),
            "mats": mats,
        }
        for c in range(NCORES)
    ]
    return bass_utils.run_bass_kernel_spmd(
        nc, in_maps, core_ids=list(range(NCORES)), **kw
    )


def kernel(tmap: np.ndarray, l_img: np.ndarray) -> np.ndarray:
    res = run_device(tmap, l_img)
    dev = sum(float(r["out"].astype(np.float64).sum()) for r in res.results)
    return np.float32(dev + _edge_contribution(tmap, l_img))


if __name__ == "__main__":
    tmap = np.random.rand(H, W).astype(np.float32)
    l_img = np.random.rand(H, W, 3).astype(np.float32)
    print(kernel(tmap, l_img))

